# revision 13
# baseline (speedup 1.0000x reference)
"""GCN (2x GCNConv + MLP head) on 8 TRN2 NeuronCores via Bass/Tile.

Scan-based aggregation (v3):
  - nodes sharded by id across 8 cores; weights replicated.
  - Per core, edges with dst in the core's shard split into 8 dst-streams
    (contiguous dst-eighths). Each GpSimd 16-partition group processes one
    stream: ap_gather pulls full 64-feat messages (feat-quad layout
    [16, W, 4] bf16, feat f = p + 16j) from a window table with a
    group-private index stream (8 streams per call in parallel).
  - Edges sorted by (window, dst). One gather call per window; gpsimd
    tensor_tensor_scan (state = mask*state + x, fp32 state) forms running
    per-dst segment sums; two small ap_gathers extract each dst's subrun
    end; presence-masked adds accumulate window partials into
    agg[128, SDPAD, 4] f32. Window tables are double-buffered so the next
    table load overlaps the current window's compute.
  - conv messages m1T/m2T built by j-split PE matmuls (casts split between
    DVE and Scalar) into DRAM [16, npad, 4] bf16. AllGather of h1d shards
    = halo exchange. MLP head in transposed space.
  - Both convs share one set of idx/mask/extraction arrays (same graph).
"""

import numpy as np
import ml_dtypes

import concourse.bass as bass  # noqa: F401
import concourse.bacc as bacc
import concourse.tile as tile
import concourse.mybir as mybir
from concourse.bass_utils import run_bass_kernel_spmd

F32 = mybir.dt.float32
BF16 = mybir.dt.bfloat16
I16 = mybir.dt.int16

NCORES = 8
NSTREAM = 8          # dst-streams per core == gpsimd 16-partition groups
WIN = 4032           # real nodes per gather window
WPAD = 64            # zero rows appended to each window table
WTBL = WIN + WPAD    # ap_gather num_elems
D = 4                # feat quad width
SDPAD = 1600         # padded dsts per stream (%16, >= ceil(12500/8))
NEH = 4              # extraction quarter-calls per window
EHALF = SDPAD // NEH  # 800 dsts per extraction call
EC = 512             # column chunk for PE phases


def wrapg(a):
    """[8, n] per-group idx streams -> [128, n//16] int16 ap_gather layout."""
    out = np.empty((128, a.shape[1] // 16), np.int16)
    for g in range(NSTREAM):
        out[16 * g:16 * g + 16] = np.ascontiguousarray(a[g].reshape(-1, 16).T)
    return out


def preprocess(n, edge_index):
    src = edge_index[0].astype(np.int64)
    dst = edge_index[1].astype(np.int64)

    deg = np.bincount(dst, minlength=n).astype(np.float64) + 1.0
    dinv = (1.0 / np.sqrt(deg)).astype(np.float32)

    shard = n // NCORES                       # 12500
    sdst = (shard + NSTREAM - 1) // NSTREAM   # 1563 dsts per stream
    nwin = (n + WIN - 1) // WIN               # 25
    npad = nwin * WIN

    loops = np.arange(n, dtype=np.int64)
    srcA = np.concatenate([src, loops])
    dstA = np.concatenate([dst, loops])

    c_of = dstA // shard
    dl = dstA - c_of * shard
    g_of = np.minimum(dl // sdst, NSTREAM - 1)
    sl = dl - g_of * sdst                     # stream-local dst
    w_of = srcA // WIN

    key = (c_of * NSTREAM + g_of) * nwin + w_of
    order = np.lexsort((sl, key))
    srcS, slS = srcA[order], sl[order]

    counts = np.bincount(key, minlength=NCORES * NSTREAM * nwin)
    starts = np.zeros(len(counts) + 1, np.int64)
    np.cumsum(counts, out=starts[1:])
    cmax = counts.reshape(NCORES, NSTREAM, nwin).max(axis=(0, 1))
    csz = ((cmax + 15) // 16) * 16            # [nwin] shared call sizes
    gtot = int(csz.sum())
    etot = nwin * SDPAD

    cores = []
    for c in range(NCORES):
        gidx = np.full((NSTREAM, gtot), WIN, np.int16)     # pad -> zero row
        mask = np.ones((NSTREAM, gtot), np.float32)
        eidx = np.zeros((NSTREAM, etot), np.int16)
        pres = np.zeros((NSTREAM, etot), np.float32)
        off = 0
        eoff = 0
        for w in range(nwin):
            cs = int(csz[w])
            for g in range(NSTREAM):
                k = (c * NSTREAM + g) * nwin + w
                a, b = starts[k], starts[k + 1]
                cnt = b - a
                d_loc = slS[a:b]
                gidx[g, off:off + cnt] = (srcS[a:b] - w * WIN).astype(np.int16)
                m = np.ones(cnt, np.float32)
                if cnt:
                    m[0] = 0.0
                    m[1:][d_loc[1:] != d_loc[:-1]] = 0.0
                    lastpos = np.zeros(SDPAD, np.int64)
                    lastpos[d_loc] = np.arange(cnt)   # last write wins
                    present = np.zeros(SDPAD, np.float32)
                    present[np.unique(d_loc)] = 1.0
                    eidx[g, eoff:eoff + SDPAD] = lastpos.astype(np.int16)
                    pres[g, eoff:eoff + SDPAD] = present
                mask[g, off:off + cnt] = m
            off += cs
            eoff += SDPAD
        dinvd = np.zeros((NSTREAM, SDPAD), np.float32)
        base = c * shard
        for g in range(NSTREAM):
            lo = base + g * sdst
            hi = min(base + min((g + 1) * sdst, shard), n)
            if lo < hi:
                dinvd[g, :hi - lo] = dinv[lo:hi]
        cores.append(dict(
            gidx=wrapg(gidx),
            mask=np.repeat(mask, 16, axis=0).astype(ml_dtypes.bfloat16),
            eidx=wrapg(eidx),
            pres=np.repeat(pres, 16, axis=0).astype(ml_dtypes.bfloat16),
            dinvd=np.repeat(dinvd, 16, axis=0),
        ))

    plan = dict(nwin=nwin, npad=npad, shard=shard, sdst=sdst,
                csz=csz, gtot=gtot, etot=etot)
    return dinv, plan, cores


def build_program(plan):
    nwin = plan["nwin"]
    npad = plan["npad"]
    shard = plan["shard"]
    sdst = plan["sdst"]
    csz = plan["csz"]
    gtot = plan["gtot"]
    etot = plan["etot"]
    dpad = SDPAD * NSTREAM
    CSMAX = int(csz.max())

    nc = bacc.Bacc("TRN2", target_bir_lowering=False, debug=False,
                   num_devices=NCORES)

    xt = nc.dram_tensor("xt", [128, npad], BF16, kind="ExternalInput")
    gidx_d = nc.dram_tensor("gidx", [128, gtot // 16], I16, kind="ExternalInput")
    mask_d = nc.dram_tensor("mask", [128, gtot], BF16, kind="ExternalInput")
    eidx_d = nc.dram_tensor("eidx", [128, etot // 16], I16, kind="ExternalInput")
    pres_d = nc.dram_tensor("pres", [128, etot], BF16, kind="ExternalInput")
    dinvd_d = nc.dram_tensor("dinvd", [128, SDPAD], F32, kind="ExternalInput")
    w1_d = nc.dram_tensor("w1", [128, 64], BF16, kind="ExternalInput")
    w2_d = nc.dram_tensor("w2", [64, 64], BF16, kind="ExternalInput")
    lw1_d = nc.dram_tensor("lw1", [64, 64], F32, kind="ExternalInput")
    lw2_d = nc.dram_tensor("lw2", [64, 32], F32, kind="ExternalInput")
    lw3_d = nc.dram_tensor("lw3", [32, 1], F32, kind="ExternalInput")
    b1q_d = nc.dram_tensor("b1q", [128, 4], F32, kind="ExternalInput")
    b2q_d = nc.dram_tensor("b2q", [128, 4], F32, kind="ExternalInput")
    lb1_d = nc.dram_tensor("lb1", [64, 1], F32, kind="ExternalInput")
    lb2_d = nc.dram_tensor("lb2", [32, 1], F32, kind="ExternalInput")
    lb3_d = nc.dram_tensor("lb3", [1, 1], F32, kind="ExternalInput")
    out_d = nc.dram_tensor("out", [1, dpad], F32, kind="ExternalOutput")

    with tile.TileContext(nc) as tc:
        with (
            tc.tile_pool(name="const", bufs=1) as cpool,
            tc.tile_pool(name="tblp", bufs=1) as tblp,
            tc.tile_pool(name="chk", bufs=1) as chk,
            tc.tile_pool(name="meta", bufs=2) as mpool,
            tc.tile_pool(name="ext", bufs=1) as epool,
            tc.tile_pool(name="epi", bufs=1) as epip,
            tc.tile_pool(name="work", bufs=2) as pool,
            tc.tile_pool(name="psA", bufs=2, space="PSUM") as psA,
            tc.tile_pool(name="psM", bufs=2, space="PSUM") as psM,
            tc.tile_pool(name="dram", bufs=1, space="DRAM") as dram,
        ):
            def load_const(dram_t, shape, dtype, tag):
                t = cpool.tile(shape, dtype, tag=tag)
                nc.sync.dma_start(t[:], dram_t[:])
                return t

            w1_t = load_const(w1_d, [128, 64], BF16, "w1")
            w2_t = load_const(w2_d, [64, 64], BF16, "w2")
            lw1_t = load_const(lw1_d, [64, 64], F32, "lw1")
            lw2_t = load_const(lw2_d, [64, 32], F32, "lw2")
            lw3_t = load_const(lw3_d, [32, 1], F32, "lw3")
            b1q_t = load_const(b1q_d, [128, 4], F32, "b1q")
            b2q_t = load_const(b2q_d, [128, 4], F32, "b2q")
            lb1_t = load_const(lb1_d, [64, 1], F32, "lb1")
            lb2_t = load_const(lb2_d, [32, 1], F32, "lb2")
            lb3_t = load_const(lb3_d, [1, 1], F32, "lb3")
            dinvd_t = load_const(dinvd_d, [128, SDPAD], F32, "dinvd")

            m1T = dram.tile([16, npad, D], BF16)
            m2T = dram.tile([16, npad, D], BF16)
            h1db = dram.tile([16, 4, dpad], BF16)
            h2b = dram.tile([16, 4, dpad], F32)
            ag_out = dram.tile([NCORES * 64, dpad], BF16, addr_space="Shared")

            tbls = [tblp.tile([128, WTBL, D], BF16, tag=f"tbl{i}",
                               name=f"tbl{i}") for i in range(2)]
            for t in tbls:
                nc.vector.memset(t[:, WIN:, :], 0.0)
            agg = tblp.tile([128, SDPAD, D], F32, tag="agg")

            def emit_msgs(lhsT_full, rhs_cols_cb, ncols, outT):
                """outT[:, o:o+cw, :] = quad-split matmul of rhs columns."""
                nchunk = (ncols + EC - 1) // EC
                for t in range(nchunk):
                    o = t * EC
                    cw = min(EC, ncols - o)
                    rhs = rhs_cols_cb(o, cw)
                    stg = pool.tile([16, EC, D], BF16, tag="stg")
                    for j in range(D):
                        ps = psA.tile([16, EC], F32, tag="psa")
                        nc.tensor.matmul(ps[:, :cw],
                                         lhsT=lhsT_full[:, 16 * j:16 * j + 16],
                                         rhs=rhs, start=True, stop=True)
                        if j % 2 == 0:
                            nc.vector.tensor_copy(stg[:, :cw, j], ps[:, :cw])
                        else:
                            nc.scalar.activation(
                                stg[:, :cw, j], ps[:, :cw],
                                mybir.ActivationFunctionType.Copy)
                    nc.sync.dma_start(outT[:, o:o + cw, :], stg[:, :cw, :])

            # --- phase A: m1T = W1^T xt (quad layout) ---
            def xt_cols(o, cw):
                st = pool.tile([128, EC], BF16, tag="xt")
                nc.sync.dma_start(st[:, :cw], xt[:, o:o + cw])
                return st[:, :cw]

            emit_msgs(w1_t, xt_cols, npad, m1T)

            def conv(msgT, out_cb):
                nc.vector.memset(agg[:], 0.0)
                goff = 0
                eoff = 0
                for w in range(nwin):
                    tbl = tbls[w % 2]
                    for g in range(NSTREAM):
                        nc.sync.dma_start(
                            tbl[16 * g:16 * g + 16, :WIN, :],
                            msgT[:, w * WIN:(w + 1) * WIN, :])
                    cs = int(csz[w])
                    it = mpool.tile([128, CSMAX // 16], I16, tag="gi")
                    nc.sync.dma_start(it[:, :cs // 16],
                                      gidx_d[:, goff // 16:(goff + cs) // 16])
                    mt = mpool.tile([128, CSMAX], BF16, tag="mk")
                    nc.sync.dma_start(mt[:, :cs], mask_d[:, goff:goff + cs])
                    gch = chk.tile([128, CSMAX, D], BF16, tag="gch")
                    nc.gpsimd.ap_gather(
                        gch[:, :cs, :], tbl[:], it[:, :cs // 16],
                        channels=128, num_elems=WTBL, d=D, num_idxs=cs)
                    for j in range(D):
                        nc.vector.tensor_tensor_scan(
                            gch[:, :cs, j], mt[:, :cs], gch[:, :cs, j],
                            0.0, op0=mybir.AluOpType.mult,
                            op1=mybir.AluOpType.add)
                    for eh in range(NEH):
                        d0 = eh * EHALF
                        et = mpool.tile([128, EHALF // 16], I16, tag="ei")
                        nc.sync.dma_start(
                            et[:],
                            eidx_d[:, (eoff + d0) // 16:(eoff + d0 + EHALF) // 16])
                        pt = mpool.tile([128, EHALF], BF16, tag="pr")
                        nc.sync.dma_start(pt[:],
                                          pres_d[:, eoff + d0:eoff + d0 + EHALF])
                        ex = epool.tile([128, EHALF, D], BF16, tag="ex")
                        nc.gpsimd.ap_gather(
                            ex[:], gch[:, :cs, :], et[:],
                            channels=128, num_elems=cs, d=D, num_idxs=EHALF)
                        exm = epool.tile([128, EHALF, D], F32, tag="exm")
                        nc.vector.tensor_tensor(
                            exm[:], ex[:],
                            pt[:].unsqueeze(2).broadcast_to([128, EHALF, D]),
                            op=mybir.AluOpType.mult)
                        nc.vector.tensor_tensor(
                            agg[:, d0:d0 + EHALF, :], agg[:, d0:d0 + EHALF, :],
                            exm[:], op=mybir.AluOpType.add)
                    goff += cs
                    eoff += SDPAD
                out_cb()

            # --- conv1 epilogue: h1d = dinv*(dinv*agg + b1) -> h1db ---
            def conv1_out():
                for eh in range(NEH):
                    d0 = eh * EHALF
                    e1 = epip.tile([128, EHALF, D], F32, tag="e1")
                    nc.vector.tensor_tensor(
                        e1[:], agg[:, d0:d0 + EHALF, :],
                        dinvd_t[:, d0:d0 + EHALF].unsqueeze(2)
                        .broadcast_to([128, EHALF, D]),
                        op=mybir.AluOpType.mult)
                    nc.vector.tensor_tensor(
                        e1[:], e1[:],
                        b1q_t[:].unsqueeze(1).broadcast_to([128, EHALF, D]),
                        op=mybir.AluOpType.add)
                    e2 = epip.tile([128, D, EHALF], BF16, tag="e2")
                    nc.vector.tensor_tensor(
                        e2[:], e1[:].transpose([0, 2, 1]),
                        dinvd_t[:, d0:d0 + EHALF].unsqueeze(1)
                        .broadcast_to([128, D, EHALF]),
                        op=mybir.AluOpType.mult)
                    for g in range(NSTREAM):
                        nc.sync.dma_start(
                            h1db[:, :, g * SDPAD + d0:g * SDPAD + d0 + EHALF],
                            e2[16 * g:16 * g + 16, :, :].opt())

            conv(m1T, conv1_out)

            nc.gpsimd.collective_compute(
                "AllGather", mybir.AluOpType.bypass,
                ins=[h1db[:].opt()],
                outs=[ag_out[:].opt()],
                replica_groups=[list(range(NCORES))],
            )

            # --- phase C: m2T = W2^T h1dT (per core, per stream segment) ---
            for c in range(NCORES):
                for g in range(NSTREAM):
                    cnt = min(sdst, shard - g * sdst)
                    node0 = c * shard + g * sdst

                    def ag_cols(o, cw, c=c, g=g):
                        st = pool.tile([64, EC], BF16, tag="agc")
                        nc.sync.dma_start(
                            st[:, :cw],
                            ag_out[c * 64:(c + 1) * 64,
                                   g * SDPAD + o:g * SDPAD + o + cw])
                        return st[:, :cw]

                    emit_msgs(w2_t, ag_cols, cnt,
                              m2T[:, node0:node0 + cnt, :])

            # --- conv2 epilogue: h2 = dinv*agg + b2 -> h2b (f32) ---
            def conv2_out():
                for eh in range(NEH):
                    d0 = eh * EHALF
                    e1 = epip.tile([128, EHALF, D], F32, tag="e1")
                    nc.vector.tensor_tensor(
                        e1[:], agg[:, d0:d0 + EHALF, :],
                        dinvd_t[:, d0:d0 + EHALF].unsqueeze(2)
                        .broadcast_to([128, EHALF, D]),
                        op=mybir.AluOpType.mult)
                    e2f = epip.tile([128, D, EHALF], F32, tag="e2f")
                    nc.vector.tensor_tensor(
                        e2f[:], e1[:].transpose([0, 2, 1]),
                        b2q_t[:].unsqueeze(2).broadcast_to([128, D, EHALF]),
                        op=mybir.AluOpType.add)
                    for g in range(NSTREAM):
                        nc.sync.dma_start(
                            h2b[:, :, g * SDPAD + d0:g * SDPAD + d0 + EHALF],
                            e2f[16 * g:16 * g + 16, :, :].opt())

            conv(m2T, conv2_out)

            # --- MLP head (transposed space, h2 streamed from DRAM) ---
            for o in range(0, dpad, EC):
                w_ = min(EC, dpad - o)
                h2c = pool.tile([64, EC], F32, tag="h2c")
                nc.sync.dma_start(h2c[:, :w_], h2b[:, :, o:o + w_])
                p1 = psM.tile([64, EC], F32, tag="mm1")
                nc.tensor.matmul(p1[:, :w_], lhsT=lw1_t[:],
                                 rhs=h2c[:, :w_], start=True, stop=True)
                z1 = pool.tile([64, EC], F32, tag="z1")
                nc.scalar.activation(z1[:, :w_], p1[:, :w_],
                                     mybir.ActivationFunctionType.Relu,
                                     bias=lb1_t[:])
                p2 = psM.tile([32, EC], F32, tag="mm2")
                nc.tensor.matmul(p2[:, :w_], lhsT=lw2_t[:], rhs=z1[:, :w_],
                                 start=True, stop=True)
                z2 = pool.tile([32, EC], F32, tag="z2")
                nc.scalar.activation(z2[:, :w_], p2[:, :w_],
                                     mybir.ActivationFunctionType.Relu,
                                     bias=lb2_t[:])
                p3 = psM.tile([1, EC], F32, tag="mm3")
                nc.tensor.matmul(p3[:, :w_], lhsT=lw3_t[:], rhs=z2[:, :w_],
                                 start=True, stop=True)
                z3 = pool.tile([1, EC], F32, tag="z3")
                nc.vector.tensor_tensor(z3[:, :w_], p3[:, :w_],
                                        lb3_t[:].broadcast_to([1, w_]),
                                        op=mybir.AluOpType.add)
                nc.sync.dma_start(out_d[:, o:o + w_], z3[:, :w_])

    nc.compile()
    return nc


def kernel(x, edge_index, W1, b1, W2, b2, lw1, lb1, lw2, lb2, lw3, lb3,
           _want_trace=False):
    x = np.asarray(x, np.float32)
    edge_index = np.asarray(edge_index)
    n = x.shape[0]

    dinv, plan, cores = preprocess(n, edge_index)
    shard, sdst, npad = plan["shard"], plan["sdst"], plan["npad"]

    xt = np.zeros((128, npad), ml_dtypes.bfloat16)
    xt[:, :n] = (x * dinv[:, None]).T.astype(ml_dtypes.bfloat16)

    # h1db/h2b row r = 4p+j holds feat p+16j -> permute consumer weight rows
    perm = np.array([(r // 4) + 16 * (r % 4) for r in range(64)])

    def quadb(b):
        # bias for [16g+p, j] = b[p + 16j]
        q = np.asarray(b, np.float32).reshape(4, 16).T  # [p, j]
        return np.ascontiguousarray(np.tile(q, (8, 1)))

    in_maps = []
    for c in range(NCORES):
        in_maps.append({
            "xt": xt,
            "gidx": cores[c]["gidx"], "mask": cores[c]["mask"],
            "eidx": cores[c]["eidx"], "pres": cores[c]["pres"],
            "dinvd": cores[c]["dinvd"],
            "w1": np.asarray(W1, np.float32).astype(ml_dtypes.bfloat16),
            "w2": np.ascontiguousarray(
                np.asarray(W2, np.float32)[perm]).astype(ml_dtypes.bfloat16),
            "lw1": np.ascontiguousarray(np.asarray(lw1, np.float32)[perm]),
            "lw2": np.ascontiguousarray(np.asarray(lw2, np.float32)),
            "lw3": np.ascontiguousarray(np.asarray(lw3, np.float32)),
            "b1q": quadb(b1), "b2q": quadb(b2),
            "lb1": np.asarray(lb1, np.float32).reshape(-1, 1),
            "lb2": np.asarray(lb2, np.float32).reshape(-1, 1),
            "lb3": np.asarray(lb3, np.float32).reshape(-1, 1),
        })

    nc = build_program(plan)
    res = run_bass_kernel_spmd(nc, in_maps, core_ids=list(range(NCORES)),
                               trace=_want_trace)
    out = np.empty((n, 1), np.float32)
    for c in range(NCORES):
        o = res.results[c]["out"][0]
        v = o.reshape(NSTREAM, SDPAD)[:, :sdst].reshape(-1)[:shard]
        out[c * shard:(c + 1) * shard, 0] = v
    kernel._last_exec_ns = res.exec_time_ns
    return out


# revision 17
# speedup vs baseline: 1.6668x; 1.6668x over previous
"""GCN (2x GCNConv + MLP head) on 8 TRN2 NeuronCores via Bass/Tile.

Scan-based aggregation:
  - nodes sharded by id across 8 cores; weights replicated.
  - Per core, edges with dst in the core's shard split into 8 dst-streams
    (contiguous dst-eighths). Each GpSimd 16-partition group processes one
    stream: ap_gather pulls full 64-feat messages (feat-quad layout
    [16, W, 4] bf16, feat f = p + 16j) from a window table with a
    group-private index stream (8 streams per call in parallel).
  - Edges sorted by (window, dst-quarter, dst). Per (window, quarter) one
    gather call; DVE tensor_tensor_scan (state = mask*state + x, fp32
    state) forms running per-dst segment sums; a small ap_gather extracts
    each dst's subrun end; presence-masked adds accumulate the window
    partials into agg[128, SDPAD, 4] f32.
  - conv messages m1T/m2T are built by j-split PE matmuls into DRAM
    [16, npad, 4] bf16; window tables load with 8 contiguous DMAs.
    AllGather of h1d shards = halo exchange. MLP head in transposed space.
  - Both convs share one set of idx/mask/extraction arrays (same graph).
"""

import numpy as np
import ml_dtypes

import concourse.bass as bass  # noqa: F401
import concourse.bacc as bacc
import concourse.tile as tile
import concourse.mybir as mybir
from concourse.bass_utils import run_bass_kernel_spmd

F32 = mybir.dt.float32
BF16 = mybir.dt.bfloat16
I16 = mybir.dt.int16

NCORES = 8
NSTREAM = 8          # dst-streams per core == gpsimd groups
WIN = 10176          # real nodes per gather window
WPAD = 64            # zero rows appended to each window table
WTBL = WIN + WPAD    # table num_elems (<= 32768/4 for d=4 bf16)
D = 4                # feat quad
NH = 4               # dst-quarter calls per window
SDPAD = 1600         # padded dsts per stream (%16, >= ceil(12500/8))
HALF = SDPAD // NH   # 400 dsts per call
EC = 512             # column chunk for PE phases


def wrapg(a):
    """[8, n] per-group idx streams -> [128, n//16] int16 ap_gather layout."""
    out = np.empty((128, a.shape[1] // 16), np.int16)
    for g in range(NSTREAM):
        out[16 * g:16 * g + 16] = np.ascontiguousarray(a[g].reshape(-1, 16).T)
    return out


def preprocess(n, edge_index):
    src = edge_index[0].astype(np.int64)
    dst = edge_index[1].astype(np.int64)

    deg = np.bincount(dst, minlength=n).astype(np.float64) + 1.0
    dinv = (1.0 / np.sqrt(deg)).astype(np.float32)

    shard = n // NCORES                       # 12500
    sdst = (shard + NSTREAM - 1) // NSTREAM   # 1563 dsts per stream
    nwin = (n + WIN - 1) // WIN               # 10
    npad = nwin * WIN

    loops = np.arange(n, dtype=np.int64)
    srcA = np.concatenate([src, loops])
    dstA = np.concatenate([dst, loops])

    c_of = dstA // shard
    dl = dstA - c_of * shard
    g_of = np.minimum(dl // sdst, NSTREAM - 1)
    sl = dl - g_of * sdst                     # stream-local dst
    w_of = srcA // WIN
    h_of = np.minimum(sl // HALF, NH - 1)

    ncalls = nwin * NH
    key = (((c_of * NSTREAM + g_of) * nwin + w_of) * NH + h_of)
    order = np.lexsort((sl, key))
    srcS, slS = srcA[order], sl[order]

    counts = np.bincount(key, minlength=NCORES * NSTREAM * ncalls)
    starts = np.zeros(len(counts) + 1, np.int64)
    np.cumsum(counts, out=starts[1:])
    cmax = counts.reshape(NCORES, NSTREAM, nwin, NH).max(axis=(0, 1))
    csz = ((cmax + 15) // 16) * 16            # [nwin, NH] shared call sizes
    gtot = int(csz.sum())
    etot = nwin * SDPAD

    cores = []
    for c in range(NCORES):
        gidx = np.full((NSTREAM, gtot), WIN, np.int16)     # pad -> zero row
        mask = np.ones((NSTREAM, gtot), np.float32)
        eidx = np.zeros((NSTREAM, etot), np.int16)
        pres = np.zeros((NSTREAM, etot), np.float32)
        off = 0
        eoff = 0
        for w in range(nwin):
            for h in range(NH):
                cs = int(csz[w, h])
                d0 = h * HALF
                for g in range(NSTREAM):
                    k = ((c * NSTREAM + g) * nwin + w) * NH + h
                    a, b = starts[k], starts[k + 1]
                    cnt = b - a
                    d_loc = slS[a:b]
                    gidx[g, off:off + cnt] = (srcS[a:b] - w * WIN).astype(np.int16)
                    m = np.ones(cnt, np.float32)
                    if cnt:
                        m[0] = 0.0
                        m[1:][d_loc[1:] != d_loc[:-1]] = 0.0
                        lastpos = np.zeros(HALF, np.int64)
                        lastpos[d_loc - d0] = np.arange(cnt)  # last write wins
                        present = np.zeros(HALF, np.float32)
                        present[np.unique(d_loc) - d0] = 1.0
                        eidx[g, eoff:eoff + HALF] = lastpos.astype(np.int16)
                        pres[g, eoff:eoff + HALF] = present
                    mask[g, off:off + cnt] = m
                off += cs
                eoff += HALF
        dinvd = np.zeros((NSTREAM, SDPAD), np.float32)
        base = c * shard
        for g in range(NSTREAM):
            lo = base + g * sdst
            hi = min(base + min((g + 1) * sdst, shard), n)
            if lo < hi:
                dinvd[g, :hi - lo] = dinv[lo:hi]
        cores.append(dict(
            gidx=wrapg(gidx),
            mask=np.repeat(mask, 16, axis=0).astype(ml_dtypes.bfloat16),
            eidx=wrapg(eidx),
            pres=np.repeat(pres, 16, axis=0).astype(ml_dtypes.bfloat16),
            dinvd=np.repeat(dinvd, 16, axis=0),
        ))

    plan = dict(nwin=nwin, npad=npad, shard=shard, sdst=sdst,
                csz=csz, gtot=gtot, etot=etot)
    return dinv, plan, cores


def build_program(plan):
    nwin = plan["nwin"]
    npad = plan["npad"]
    shard = plan["shard"]
    sdst = plan["sdst"]
    csz = plan["csz"]
    gtot = plan["gtot"]
    etot = plan["etot"]
    dpad = SDPAD * NSTREAM
    CSMAX = int(csz.max())

    nc = bacc.Bacc("TRN2", target_bir_lowering=False, debug=False,
                   num_devices=NCORES)

    xt = nc.dram_tensor("xt", [128, npad], BF16, kind="ExternalInput")
    gidx_d = nc.dram_tensor("gidx", [128, gtot // 16], I16, kind="ExternalInput")
    mask_d = nc.dram_tensor("mask", [128, gtot], BF16, kind="ExternalInput")
    eidx_d = nc.dram_tensor("eidx", [128, etot // 16], I16, kind="ExternalInput")
    pres_d = nc.dram_tensor("pres", [128, etot], BF16, kind="ExternalInput")
    dinvd_d = nc.dram_tensor("dinvd", [128, SDPAD], F32, kind="ExternalInput")
    w1_d = nc.dram_tensor("w1", [128, 64], BF16, kind="ExternalInput")
    w2_d = nc.dram_tensor("w2", [64, 64], BF16, kind="ExternalInput")
    lw1_d = nc.dram_tensor("lw1", [64, 64], F32, kind="ExternalInput")
    lw2_d = nc.dram_tensor("lw2", [64, 32], F32, kind="ExternalInput")
    lw3_d = nc.dram_tensor("lw3", [32, 1], F32, kind="ExternalInput")
    b1q_d = nc.dram_tensor("b1q", [128, 4], F32, kind="ExternalInput")
    b2q_d = nc.dram_tensor("b2q", [128, 4], F32, kind="ExternalInput")
    lb1_d = nc.dram_tensor("lb1", [64, 1], F32, kind="ExternalInput")
    lb2_d = nc.dram_tensor("lb2", [32, 1], F32, kind="ExternalInput")
    lb3_d = nc.dram_tensor("lb3", [1, 1], F32, kind="ExternalInput")
    out_d = nc.dram_tensor("out", [1, dpad], F32, kind="ExternalOutput")

    with tile.TileContext(nc) as tc:
        with (
            tc.tile_pool(name="const", bufs=1) as cpool,
            tc.tile_pool(name="tblp", bufs=1) as tblp,
            tc.tile_pool(name="chk", bufs=2) as chk,
            tc.tile_pool(name="meta", bufs=2) as mpool,
            tc.tile_pool(name="ext", bufs=1) as epool,
            tc.tile_pool(name="epi", bufs=1) as epip,
            tc.tile_pool(name="work", bufs=2) as pool,
            tc.tile_pool(name="psA", bufs=2, space="PSUM") as psA,
            tc.tile_pool(name="psM", bufs=2, space="PSUM") as psM,
            tc.tile_pool(name="dram", bufs=1, space="DRAM") as dram,
        ):
            def load_const(dram_t, shape, dtype, tag):
                t = cpool.tile(shape, dtype, tag=tag)
                nc.sync.dma_start(t[:], dram_t[:])
                return t

            w1_t = load_const(w1_d, [128, 64], BF16, "w1")
            w2_t = load_const(w2_d, [64, 64], BF16, "w2")
            lw1_t = load_const(lw1_d, [64, 64], F32, "lw1")
            lw2_t = load_const(lw2_d, [64, 32], F32, "lw2")
            lw3_t = load_const(lw3_d, [32, 1], F32, "lw3")
            b1q_t = load_const(b1q_d, [128, 4], F32, "b1q")
            b2q_t = load_const(b2q_d, [128, 4], F32, "b2q")
            lb1_t = load_const(lb1_d, [64, 1], F32, "lb1")
            lb2_t = load_const(lb2_d, [32, 1], F32, "lb2")
            lb3_t = load_const(lb3_d, [1, 1], F32, "lb3")
            dinvd_t = load_const(dinvd_d, [128, SDPAD], F32, "dinvd")

            m1T = dram.tile([16, npad, D], BF16)
            m2T = dram.tile([16, npad, D], BF16)
            h1db = dram.tile([16, 4, dpad], BF16)
            h2b = dram.tile([16, 4, dpad], F32)
            ag_out = dram.tile([NCORES * 64, dpad], BF16, addr_space="Shared")

            tbl = tblp.tile([128, WTBL, D], BF16, tag="tbl")
            nc.vector.memset(tbl[:, WIN:, :], 0.0)
            agg = tblp.tile([128, SDPAD, D], F32, tag="agg")

            def emit_msgs(lhsT_full, rhs_cols_cb, ncols, outT):
                """outT[:, o:o+cw, :] = quad-split matmul of rhs columns."""
                nchunk = (ncols + EC - 1) // EC
                for t in range(nchunk):
                    o = t * EC
                    cw = min(EC, ncols - o)
                    rhs = rhs_cols_cb(o, cw)
                    stg = pool.tile([16, EC, D], BF16, tag="stg")
                    for j in range(D):
                        ps = psA.tile([16, EC], F32, tag="psa")
                        nc.tensor.matmul(ps[:, :cw],
                                         lhsT=lhsT_full[:, 16 * j:16 * j + 16],
                                         rhs=rhs, start=True, stop=True)
                        nc.vector.tensor_copy(stg[:, :cw, j], ps[:, :cw])
                    nc.sync.dma_start(outT[:, o:o + cw, :], stg[:, :cw, :])

            # --- phase A: m1T = W1^T xt (quad layout) ---
            def xt_cols(o, cw):
                st = pool.tile([128, EC], BF16, tag="xt")
                nc.sync.dma_start(st[:, :cw], xt[:, o:o + cw])
                return st[:, :cw]

            emit_msgs(w1_t, xt_cols, npad, m1T)

            def conv(msgT, out_cb):
                nc.vector.memset(agg[:], 0.0)
                goff = 0
                eoff = 0
                for w in range(nwin):
                    for g in range(NSTREAM):
                        nc.sync.dma_start(
                            tbl[16 * g:16 * g + 16, :WIN, :],
                            msgT[:, w * WIN:(w + 1) * WIN, :])
                    for h in range(NH):
                        cs = int(csz[w, h])
                        it = mpool.tile([128, CSMAX // 16], I16, tag="gi")
                        nc.sync.dma_start(it[:, :cs // 16],
                                          gidx_d[:, goff // 16:(goff + cs) // 16])
                        mt = mpool.tile([128, CSMAX], BF16, tag="mk")
                        nc.sync.dma_start(mt[:, :cs], mask_d[:, goff:goff + cs])
                        gch = chk.tile([128, CSMAX, D], BF16, tag="gch")
                        nc.gpsimd.ap_gather(
                            gch[:, :cs, :], tbl[:], it[:, :cs // 16],
                            channels=128, num_elems=WTBL, d=D, num_idxs=cs)
                        for j in range(D):
                            nc.vector.tensor_tensor_scan(
                                gch[:, :cs, j], mt[:, :cs], gch[:, :cs, j],
                                0.0, op0=mybir.AluOpType.mult,
                                op1=mybir.AluOpType.add)
                        et = mpool.tile([128, HALF // 16], I16, tag="ei")
                        nc.sync.dma_start(
                            et[:],
                            eidx_d[:, eoff // 16:(eoff + HALF) // 16])
                        pt = mpool.tile([128, HALF], BF16, tag="pr")
                        nc.sync.dma_start(pt[:],
                                          pres_d[:, eoff:eoff + HALF])
                        ex = epool.tile([128, HALF, D], BF16, tag="ex")
                        nc.gpsimd.ap_gather(
                            ex[:], gch[:, :cs, :], et[:],
                            channels=128, num_elems=cs, d=D, num_idxs=HALF)
                        exm = epool.tile([128, HALF, D], F32, tag="exm")
                        nc.vector.tensor_tensor(
                            exm[:], ex[:],
                            pt[:].unsqueeze(2).broadcast_to([128, HALF, D]),
                            op=mybir.AluOpType.mult)
                        d0 = h * HALF
                        nc.vector.tensor_tensor(
                            agg[:, d0:d0 + HALF, :], agg[:, d0:d0 + HALF, :],
                            exm[:], op=mybir.AluOpType.add)
                        goff += cs
                        eoff += HALF
                out_cb()

            # --- conv1 epilogue: h1d = dinv*(dinv*agg + b1) -> h1db ---
            def conv1_out():
                for h in range(NH):
                    d0 = h * HALF
                    e1 = epip.tile([128, HALF, D], F32, tag="e1")
                    nc.vector.tensor_tensor(
                        e1[:], agg[:, d0:d0 + HALF, :],
                        dinvd_t[:, d0:d0 + HALF].unsqueeze(2)
                        .broadcast_to([128, HALF, D]),
                        op=mybir.AluOpType.mult)
                    nc.vector.tensor_tensor(
                        e1[:], e1[:],
                        b1q_t[:].unsqueeze(1).broadcast_to([128, HALF, D]),
                        op=mybir.AluOpType.add)
                    e2 = epip.tile([128, D, HALF], BF16, tag="e2")
                    nc.vector.tensor_tensor(
                        e2[:], e1[:].transpose([0, 2, 1]),
                        dinvd_t[:, d0:d0 + HALF].unsqueeze(1)
                        .broadcast_to([128, D, HALF]),
                        op=mybir.AluOpType.mult)
                    for g in range(NSTREAM):
                        nc.sync.dma_start(
                            h1db[:, :, g * SDPAD + d0:g * SDPAD + d0 + HALF],
                            e2[16 * g:16 * g + 16, :, :].opt())

            conv(m1T, conv1_out)

            nc.gpsimd.collective_compute(
                "AllGather", mybir.AluOpType.bypass,
                ins=[h1db[:].opt()],
                outs=[ag_out[:].opt()],
                replica_groups=[list(range(NCORES))],
            )

            # --- phase C: m2T = W2^T h1dT (per core, per stream segment) ---
            for c in range(NCORES):
                for g in range(NSTREAM):
                    cnt = min(sdst, shard - g * sdst)
                    node0 = c * shard + g * sdst

                    def ag_cols(o, cw, c=c, g=g):
                        st = pool.tile([64, EC], BF16, tag="agc")
                        nc.sync.dma_start(
                            st[:, :cw],
                            ag_out[c * 64:(c + 1) * 64,
                                   g * SDPAD + o:g * SDPAD + o + cw])
                        return st[:, :cw]

                    emit_msgs(w2_t, ag_cols, cnt,
                              m2T[:, node0:node0 + cnt, :])

            # --- conv2 epilogue: h2 = dinv*agg + b2 -> h2b (f32) ---
            def conv2_out():
                for h in range(NH):
                    d0 = h * HALF
                    e1 = epip.tile([128, HALF, D], F32, tag="e1")
                    nc.vector.tensor_tensor(
                        e1[:], agg[:, d0:d0 + HALF, :],
                        dinvd_t[:, d0:d0 + HALF].unsqueeze(2)
                        .broadcast_to([128, HALF, D]),
                        op=mybir.AluOpType.mult)
                    e2f = epip.tile([128, D, HALF], F32, tag="e2f")
                    nc.vector.tensor_tensor(
                        e2f[:], e1[:].transpose([0, 2, 1]),
                        b2q_t[:].unsqueeze(2).broadcast_to([128, D, HALF]),
                        op=mybir.AluOpType.add)
                    for g in range(NSTREAM):
                        nc.sync.dma_start(
                            h2b[:, :, g * SDPAD + d0:g * SDPAD + d0 + HALF],
                            e2f[16 * g:16 * g + 16, :, :].opt())

            conv(m2T, conv2_out)

            # --- MLP head (transposed space, h2 streamed from DRAM) ---
            for o in range(0, dpad, EC):
                w_ = min(EC, dpad - o)
                h2c = pool.tile([64, EC], F32, tag="h2c")
                nc.sync.dma_start(h2c[:, :w_], h2b[:, :, o:o + w_])
                p1 = psM.tile([64, EC], F32, tag="mm1")
                nc.tensor.matmul(p1[:, :w_], lhsT=lw1_t[:],
                                 rhs=h2c[:, :w_], start=True, stop=True)
                z1 = pool.tile([64, EC], F32, tag="z1")
                nc.scalar.activation(z1[:, :w_], p1[:, :w_],
                                     mybir.ActivationFunctionType.Relu,
                                     bias=lb1_t[:])
                p2 = psM.tile([32, EC], F32, tag="mm2")
                nc.tensor.matmul(p2[:, :w_], lhsT=lw2_t[:], rhs=z1[:, :w_],
                                 start=True, stop=True)
                z2 = pool.tile([32, EC], F32, tag="z2")
                nc.scalar.activation(z2[:, :w_], p2[:, :w_],
                                     mybir.ActivationFunctionType.Relu,
                                     bias=lb2_t[:])
                p3 = psM.tile([1, EC], F32, tag="mm3")
                nc.tensor.matmul(p3[:, :w_], lhsT=lw3_t[:], rhs=z2[:, :w_],
                                 start=True, stop=True)
                z3 = pool.tile([1, EC], F32, tag="z3")
                nc.vector.tensor_tensor(z3[:, :w_], p3[:, :w_],
                                        lb3_t[:].broadcast_to([1, w_]),
                                        op=mybir.AluOpType.add)
                nc.sync.dma_start(out_d[:, o:o + w_], z3[:, :w_])

    nc.compile()
    return nc


def kernel(x, edge_index, W1, b1, W2, b2, lw1, lb1, lw2, lb2, lw3, lb3,
           _want_trace=False):
    x = np.asarray(x, np.float32)
    edge_index = np.asarray(edge_index)
    n = x.shape[0]

    dinv, plan, cores = preprocess(n, edge_index)
    shard, sdst, npad = plan["shard"], plan["sdst"], plan["npad"]

    xt = np.zeros((128, npad), ml_dtypes.bfloat16)
    xt[:, :n] = (x * dinv[:, None]).T.astype(ml_dtypes.bfloat16)

    # h1db/h2b row r = 4p+j holds feat p+16j -> permute consumer weight rows
    perm = np.array([(r // 4) + 16 * (r % 4) for r in range(64)])

    def quadb(b):
        # bias for [16g+p, j] = b[p + 16j]
        q = np.asarray(b, np.float32).reshape(4, 16).T  # [p, j]
        return np.ascontiguousarray(np.tile(q, (8, 1)))

    in_maps = []
    for c in range(NCORES):
        in_maps.append({
            "xt": xt,
            "gidx": cores[c]["gidx"], "mask": cores[c]["mask"],
            "eidx": cores[c]["eidx"], "pres": cores[c]["pres"],
            "dinvd": cores[c]["dinvd"],
            "w1": np.asarray(W1, np.float32).astype(ml_dtypes.bfloat16),
            "w2": np.ascontiguousarray(
                np.asarray(W2, np.float32)[perm]).astype(ml_dtypes.bfloat16),
            "lw1": np.ascontiguousarray(np.asarray(lw1, np.float32)[perm]),
            "lw2": np.ascontiguousarray(np.asarray(lw2, np.float32)),
            "lw3": np.ascontiguousarray(np.asarray(lw3, np.float32)),
            "b1q": quadb(b1), "b2q": quadb(b2),
            "lb1": np.asarray(lb1, np.float32).reshape(-1, 1),
            "lb2": np.asarray(lb2, np.float32).reshape(-1, 1),
            "lb3": np.asarray(lb3, np.float32).reshape(-1, 1),
        })

    nc = build_program(plan)
    res = run_bass_kernel_spmd(nc, in_maps, core_ids=list(range(NCORES)),
                               trace=_want_trace)
    out = np.empty((n, 1), np.float32)
    for c in range(NCORES):
        o = res.results[c]["out"][0]
        v = o.reshape(NSTREAM, SDPAD)[:, :sdst].reshape(-1)[:shard]
        out[c * shard:(c + 1) * shard, 0] = v
    kernel._last_exec_ns = res.exec_time_ns
    return out
